# revision 1
# baseline (speedup 1.0000x reference)
"""v6: [128,256] packing — 2 partitions per token, halved free dim.

Host gives shifted indices (xt' = xt - 256*parity) so each partition
compares against a plain 0..255 iota for its half of the vocab. Pair
sums (s[x_t] one-hot dot, row sum) go through a tiny K=128 N=1 PE
matmul with a block-pair matrix, broadcasting back to both partitions.

Chain reorder: u = relu(a2*(q*rec) - b*(1 - eps*rec)) using pden*rec==1,
so the per-token scalars (from the pair-summed one-hot dot) are only
needed late in the DVE stream.
"""
import numpy as np
from contextlib import ExitStack

N = 512
V = 512
NCORES = 8
NT = N // NCORES  # 64 tokens/core
P = 2 * NT        # 128 partitions
H = V // 2        # 256 free
EPS = 1e-8


def build_default():
    import concourse.bass as bass
    import concourse.mybir as mybir
    from concourse import bacc
    from concourse import tile

    fp32 = mybir.dt.float32
    Alu = mybir.AluOpType
    Act = mybir.ActivationFunctionType

    nc = bacc.Bacc("TRN2", target_bir_lowering=False, debug=False)

    W = 4 + H + H + P  # packed input width: sm | io2 | s2 | mm
    pk_d = nc.dram_tensor("pk", [P, W], fp32, kind="ExternalInput")
    out_d = nc.dram_tensor("out", [NT, V], fp32, kind="ExternalOutput")

    with tile.TileContext(nc) as tc, ExitStack() as ctx:
        pool = ctx.enter_context(tc.tile_pool(name="main", bufs=1))
        psum = ctx.enter_context(tc.tile_pool(name="ps", bufs=1, space="PSUM"))

        def big(tag, dt=fp32):
            return pool.tile([P, H], dt, name=tag, tag=tag)

        def small(tag, dt=fp32):
            return pool.tile([P, 1], dt, name=tag, tag=tag)

        pk_t = pool.tile([P, W], fp32, name="pk_t")

        # ACT func-table prewarm overlapping the DMA front
        warm = pool.tile([1, 1], fp32, name="warm")
        nc.gpsimd.memset(warm[:], 0.0)
        nc.scalar.activation(warm[:], warm[:], Act.Copy, bias=0.0)

        nc.sync.dma_start(pk_t[:], pk_d.ap())

        xt_c, x1_c = pk_t[:, 0:1], pk_t[:, 1:2]
        k_c, dk_c = pk_t[:, 2:3], pk_t[:, 3:4]
        io_t = pk_t[:, 4 : 4 + H]
        s_t = pk_t[:, 4 + H : 4 + 2 * H]
        m_t = pk_t[:, 4 + 2 * H : 4 + 2 * H + P]

        # s + eps on ACT (off the DVE stream)
        se_t = big("se_t")
        nc.scalar.activation(se_t[:], s_t, Act.Copy, bias=EPS)

        # DVE stream, in emission order
        dc, di, junk = big("dc"), big("di"), big("junk")
        sxth = small("sxth")
        nc.vector.tensor_scalar(dc[:], io_t, x1_c, None, Alu.is_equal)
        nc.vector.tensor_scalar(di[:], io_t, xt_c, None, Alu.is_equal)
        nc.vector.tensor_tensor(junk[:], di[:], s_t, Alu.mult)
        nc.vector.tensor_reduce(sxth[:], junk[:], mybir.AxisListType.X, Alu.add)

        # pair-sum s_xt across the partition pair on PE, copy back to SBUF
        sxt_p = psum.tile([P, 1], fp32, name="sxt_p")
        s_xt = small("s_xt")
        nc.tensor.matmul(sxt_p[:], lhsT=m_t, rhs=sxth[:], start=True, stop=True)
        nc.scalar.activation(s_xt[:], sxt_p[:], Act.Copy, bias=0.0)

        # per-token scalars: omk/t_as on ACT (idle), rest inline on DVE later
        # (gpsimd tensor ops crash the neuronxcc compile — keep Pool DMA-free)
        eq = small("eq")
        omk = small("omk")
        t_as = small("t_as")
        a1 = small("a1")
        b_t = small("b_t")
        nc.scalar.activation(omk[:], k_c, Act.Copy, scale=-1.0, bias=1.0)
        nc.scalar.activation(t_as[:], s_xt[:], Act.Copy, scale=omk[:], bias=0.0)

        # main chain
        q, kq, pden, rec = big("q"), big("kq"), big("pden"), big("rec")
        v_t, g, y, z = big("v_t"), big("g"), big("y"), big("z")
        e, u = big("e"), big("u")
        rsh = small("rsh")
        nc.vector.tensor_tensor(q[:], dc[:], s_t, Alu.subtract)
        nc.vector.tensor_scalar(kq[:], q[:], k_c, None, Alu.mult)
        nc.vector.tensor_tensor(pden[:], kq[:], se_t[:], Alu.add)
        nc.vector.reciprocal(rec[:], pden[:])
        nc.vector.tensor_scalar(v_t[:], rec[:], -EPS, 1.0, Alu.mult, Alu.add)
        nc.vector.tensor_tensor(g[:], q[:], rec[:], Alu.mult)
        nc.vector.tensor_tensor(eq[:], xt_c, x1_c, Alu.is_equal)
        nc.vector.tensor_scalar(a1[:], eq[:], k_c, t_as[:], Alu.mult, Alu.add)
        nc.vector.tensor_scalar(b_t[:], eq[:], s_xt[:], dk_c, Alu.subtract, Alu.mult)
        nc.vector.tensor_scalar(y[:], g[:], a1[:], dk_c, Alu.mult, Alu.mult)
        nc.vector.tensor_scalar(z[:], v_t[:], b_t[:], None, Alu.mult)
        nc.vector.tensor_tensor(e[:], y[:], z[:], Alu.subtract)
        nc.vector.tensor_scalar(u[:], e[:], 0.0, None, Alu.max)
        nc.vector.tensor_reduce(rsh[:], u[:], mybir.AxisListType.X, Alu.add)

        # pair-sum row sums on PE
        rs_p = psum.tile([P, 1], fp32, name="rs_p")
        rowsum = small("rowsum")
        nc.tensor.matmul(rs_p[:], lhsT=m_t, rhs=rsh[:], start=True, stop=True)
        nc.scalar.activation(rowsum[:], rs_p[:], Act.Copy, bias=0.0)

        t3, row = big("t3"), big("row")
        nc.vector.tensor_scalar(t3[:], di[:], rowsum[:], None, Alu.mult)
        nc.vector.tensor_tensor(row[:], u[:], t3[:], Alu.subtract)

        out_ap = out_d.ap().rearrange("a (h b) -> (a h) b", h=2)
        nc.sync.dma_start(out_ap, row[:])

    nc.compile()
    return nc


def in_maps(source_p, k_t, d_k_t, x_t, x_1):
    s = np.asarray(source_p, dtype=np.float32).reshape(V)
    kf = np.float32(np.asarray(k_t).reshape(()))
    dkf = np.float32(np.asarray(d_k_t).reshape(()))
    xt = np.asarray(x_t).reshape(N).astype(np.int64)
    x1 = np.asarray(x_1).reshape(N).astype(np.int64)

    W = 4 + H + H + P
    parity = np.tile(np.array([0, 1], dtype=np.int64), NT)  # per partition

    base = np.empty((P, W), dtype=np.float32)
    base[:, 4 : 4 + H] = np.arange(H, dtype=np.float32)[None, :]
    base[0::2, 4 + H : 4 + 2 * H] = s[:H]
    base[1::2, 4 + H : 4 + 2 * H] = s[H:]
    base[:, 4 + 2 * H :] = np.kron(
        np.eye(NT, dtype=np.float32), np.ones((2, 2), dtype=np.float32)
    )
    base[:, 2] = kf
    base[:, 3] = dkf

    maps = []
    for c in range(NCORES):
        lo, hi = c * NT, (c + 1) * NT
        pk = base.copy()
        pk[:, 0] = (np.repeat(xt[lo:hi], 2) - H * parity).astype(np.float32)
        pk[:, 1] = (np.repeat(x1[lo:hi], 2) - H * parity).astype(np.float32)
        maps.append({"pk": pk})
    return maps


_CACHE = {}


def _get_nc():
    if "nc" not in _CACHE:
        _CACHE["nc"] = build_default()
    return _CACHE["nc"]


def _in_maps(source_p, k_t, d_k_t, x_t, x_1):
    return in_maps(source_p, k_t, d_k_t, x_t, x_1)


def kernel(source_p, k_t, d_k_t, x_t, x_1):
    from concourse.bass_utils import run_bass_kernel_spmd

    nc = _get_nc()
    maps = in_maps(source_p, k_t, d_k_t, x_t, x_1)
    res = run_bass_kernel_spmd(nc, maps, list(range(NCORES)))
    out = np.concatenate([res.results[c]["out"] for c in range(NCORES)], axis=0)
    return out.astype(np.float32)



# revision 2
# speedup vs baseline: 1.4878x; 1.4878x over previous
"""v7: closed-form sparse kinetic-optimal row.

Math: for i = x_t[n], numer(j) = p(i)*pdot(j) - pdot(i)*p(j) is EXACTLY 0
for j not in {x_t, x_1}; the full rate row reduces to
    val = max(dk*s[xt], 0) / ((1-k)*s[x1] + k + eps)
    row[x1] += val ; row[xt] -= val
(verified vs reference: rel err ~1e-6).

Device work per core (64 tokens, [128,256] two-partitions-per-token):
  - gpsimd iota (no iota DMA)
  - two fused scalar_tensor_tensor gathers (compare*s + free-dim accum)
  - stream_shuffle pair-swap + add for the cross-partition pair sum
    (replaces the PE matmul + pair-matrix DMA of v6)
  - small-op val math, one-hot difference, scale, dense out DMA.
"""
import numpy as np
from contextlib import ExitStack

N = 512
V = 512
NCORES = 8
NT = N // NCORES  # 64 tokens/core
P = 2 * NT        # 128 partitions
H = V // 2        # 256 free
EPS = 1e-8

USE_IOTA = True

# pair-swap within each 32-partition quadrant
SWAP_MASK = [i ^ 1 for i in range(32)]


def build_default():
    import concourse.bass as bass
    import concourse.mybir as mybir
    from concourse import bacc
    from concourse import tile

    fp32 = mybir.dt.float32
    Alu = mybir.AluOpType

    nc = bacc.Bacc("TRN2", target_bir_lowering=False, debug=False)

    W = 8 + H + (0 if USE_IOTA else H)
    pk_d = nc.dram_tensor("pk", [P, W], fp32, kind="ExternalInput")
    out_d = nc.dram_tensor("out", [NT, V], fp32, kind="ExternalOutput")

    with tile.TileContext(nc) as tc, ExitStack() as ctx:
        pool = ctx.enter_context(tc.tile_pool(name="main", bufs=1))

        pk_t = pool.tile([P, W], fp32, name="pk_t")
        if USE_IOTA:
            io_t = pool.tile([P, H], fp32, name="io_t")
            nc.gpsimd.iota(
                io_t[:],
                pattern=[[1, H]],
                base=0,
                channel_multiplier=0,
                allow_small_or_imprecise_dtypes=True,
            )

        nc.sync.dma_start(pk_t[:], pk_d.ap())

        x1s, xts = pk_t[:, 0:1], pk_t[:, 1:2]
        omk_c, kpe_c = pk_t[:, 2:3], pk_t[:, 3:4]
        s_t = pk_t[:, 8 : 8 + H]
        if not USE_IOTA:
            io_t = pk_t[:, 8 + H : 8 + 2 * H]

        def big(tag):
            return pool.tile([P, H], fp32, name=tag, tag=tag)

        def small(tag, w=1):
            return pool.tile([P, w], fp32, name=tag, tag=tag)

        g1, g2 = big("g1"), big("g2")
        rh = small("rh", 2)
        # gather partials: rh[:,0] = sum (io==x1')*s_half ; rh[:,1] same for xt
        nc.vector.scalar_tensor_tensor(
            g1[:], io_t[:], x1s, s_t, Alu.is_equal, Alu.mult, accum_out=rh[:, 0:1]
        )
        nc.vector.scalar_tensor_tensor(
            g2[:], io_t[:], xts, s_t, Alu.is_equal, Alu.mult, accum_out=rh[:, 1:2]
        )

        # one-hot difference d = (io==x1') - (io==xt') while the pair math runs
        di, d = big("di"), big("d")
        nc.vector.tensor_scalar(di[:], io_t[:], xts, None, Alu.is_equal)
        nc.vector.scalar_tensor_tensor(
            d[:], io_t[:], x1s, di[:], Alu.is_equal, Alu.subtract
        )

        # cross-partition pair sum via stream shuffle (partition p <-> p^1)
        rsw, prs = small("rsw", 2), small("prs", 2)
        nc.vector.stream_shuffle(rsw[:], rh[:], SWAP_MASK)
        nc.vector.tensor_tensor(prs[:], rh[:], rsw[:], Alu.add)

        # val = relu(s_xt * 1/((1-k)/dk * s_x1 + (k+eps)/dk))
        den, rec, val = small("den"), small("rec"), small("val")
        nc.vector.tensor_scalar(den[:], prs[:, 0:1], omk_c, kpe_c, Alu.mult, Alu.add)
        nc.vector.reciprocal(rec[:], den[:])
        nc.vector.tensor_scalar(val[:], prs[:, 1:2], rec[:], 0.0, Alu.mult, Alu.max)

        row = big("row")
        nc.vector.tensor_scalar(row[:], d[:], val[:], None, Alu.mult)

        out_ap = out_d.ap().rearrange("a (h b) -> (a h) b", h=2)
        nc.sync.dma_start(out_ap, row[:])

    nc.compile()
    return nc


def in_maps(source_p, k_t, d_k_t, x_t, x_1):
    s = np.asarray(source_p, dtype=np.float32).reshape(V)
    kf = float(np.asarray(k_t).reshape(()))
    dkf = float(np.asarray(d_k_t).reshape(()))
    xt = np.asarray(x_t).reshape(N).astype(np.int64)
    x1 = np.asarray(x_1).reshape(N).astype(np.int64)

    W = 8 + H + (0 if USE_IOTA else H)
    parity = np.tile(np.array([0, 1], dtype=np.int64), NT)

    base = np.zeros((P, W), dtype=np.float32)
    base[0::2, 8 : 8 + H] = s[:H]
    base[1::2, 8 : 8 + H] = s[H:]
    if not USE_IOTA:
        base[:, 8 + H : 8 + 2 * H] = np.arange(H, dtype=np.float32)[None, :]
    with np.errstate(divide="ignore"):
        base[:, 2] = np.float32((1.0 - kf) / dkf)
        base[:, 3] = np.float32((kf + EPS) / dkf)

    maps = []
    for c in range(NCORES):
        lo, hi = c * NT, (c + 1) * NT
        pk = base.copy()
        pk[:, 0] = (np.repeat(x1[lo:hi], 2) - H * parity).astype(np.float32)
        pk[:, 1] = (np.repeat(xt[lo:hi], 2) - H * parity).astype(np.float32)
        maps.append({"pk": pk})
    return maps


_CACHE = {}


def _get_nc():
    if "nc" not in _CACHE:
        _CACHE["nc"] = build_default()
    return _CACHE["nc"]


def _in_maps(source_p, k_t, d_k_t, x_t, x_1):
    return in_maps(source_p, k_t, d_k_t, x_t, x_1)


def kernel(source_p, k_t, d_k_t, x_t, x_1):
    from concourse.bass_utils import run_bass_kernel_spmd

    nc = _get_nc()
    maps = in_maps(source_p, k_t, d_k_t, x_t, x_1)
    res = run_bass_kernel_spmd(nc, maps, list(range(NCORES)))
    out = np.concatenate([res.results[c]["out"] for c in range(NCORES)], axis=0)
    return out.astype(np.float32)
